# revision 8
# baseline (speedup 1.0000x reference)
"""Trainium2 Bass kernel for a 4-bit-quantized linear layer.

Computes y = x @ W^T + bias where W [O, I] is dequantized on-chip from
packed int4 nibbles with a per-group (16 along I) abs-max scale:
    W[o, i] = (q/15) * 2*norm - norm = (2*norm/15) * (q - 7.5)

Distribution: tensor-parallel over output features. Each of the 8 cores
owns O/8 = 2048 output rows (its slice of quantized_weights /
weight_normalization / bias), the input is replicated, and the host
concatenates the per-core [T, 2048] outputs along the feature axis.

Per-core device program:
  1. Dequantize the weight shard to fp16 in [o, i] layout on DVE
     (bitwise nibble extraction + fused (q - 7.5) * a with a = 2*norm/15),
     then DMA-transpose to a resident [i, o] fp16 tensor in SBUF.
  2. Stream x in 128-token blocks: fp32->fp16 cast during the (SWDGE) DMA
     load, DMA-transpose to x^T k-tiles, and accumulate
     psum[t, o] += xT_k^T @ WT_k over the 32 k-tiles on the TensorEngine.
  3. Add bias (DVE, fused with the PSUM->SBUF copy) and DMA out.
"""

import numpy as np

import concourse.bass as bass
import concourse.mybir as mybir
from concourse import bacc
from concourse.tile import TileContext

# Full problem shapes (hardcoded; kernel.py must be self-contained).
B, S = 4, 2048
IN_F = 4096
OUT_F = 16384
GROUP = 16
N_CORES = 8
T_FULL = B * S                    # 8192 tokens
O_SH = OUT_F // N_CORES           # 2048 output features per core
G_SH = O_SH * IN_F // GROUP       # 524288 quant groups per core

F16 = mybir.dt.float16
F32 = mybir.dt.float32
I32 = mybir.dt.int32

P = 128  # partitions


def emit_linear4bit(tc, x_ap, qw_ap, wn_ap, bias_ap, y_ap, T, I, O):
    """Emit the per-core program into TileContext tc.

    x:  [T, I]  f32   (replicated input)
    qw: [O*I/16, 8] i32  (this core's group rows; each i32 holds one byte
                          = two nibbles)
    wn: [O*I/16, 1] f32  (per-group scale)
    bias: [O] f32
    y:  [T, O] f32
    """
    nc = tc.nc
    op = mybir.AluOpType

    KT = I // P                   # k tiles (contraction)
    OT = O // P                   # o tiles for dequant
    OC = min(512, O)              # psum chunk along o
    NOC = O // OC
    TB = T // P                   # token blocks
    GPI = I // GROUP              # groups per output row
    BPG = GROUP // 2              # bytes per group (8)

    # ---- resident tensors -------------------------------------------------
    # W^T, fp16, [i (partition within k-tile), k-tile, o] -> 2*KT*O bytes/part
    singles = tc.alloc_tile_pool(name="singles", bufs=1)
    wT = singles.tile([P, KT, O], F16, tag="wT", name="wT")
    bias_rep = singles.tile([P, O], F32, tag="bias_rep", name="bias_rep")

    # bias replicated across partitions via a broadcast-source DMA
    nc.sync.dma_start(bias_rep[:, :], bias_ap[None, :].broadcast_to([P, O]))

    # views of the weight inputs grouped by 128-row o tiles
    # qw rows: g = o * GPI + gi ; o = ot*128 + p
    qw_r = qw_ap.rearrange("(ot p g) b -> ot p (g b)", ot=OT, p=P, g=GPI)
    wn_r = wn_ap.rearrange("(ot p g) one -> ot p (g one)", ot=OT, p=P, g=GPI)

    # ---- phase 1: dequantize + transpose W --------------------------------
    with (
        tc.tile_pool(name="qpool", bufs=2) as qpool,
        tc.tile_pool(name="spool", bufs=2) as spool,
        tc.tile_pool(name="nibpool", bufs=2) as nibpool,
        tc.tile_pool(name="wdpool", bufs=2) as wdpool,
    ):
        for ot in range(OT):
            qt = qpool.tile([P, GPI * BPG], I32, tag="qt")
            nc.sync.dma_start(qt[:, :], qw_r[ot])
            wnt = spool.tile([P, GPI], F32, tag="wnt")
            nc.sync.dma_start(wnt[:, :], wn_r[ot])
            at = spool.tile([P, GPI], F32, tag="at")
            nc.vector.tensor_scalar_mul(at[:, :], wnt[:, :], 2.0 / 15.0)

            lo = nibpool.tile([P, GPI * BPG], I32, tag="nib")
            hi = nibpool.tile([P, GPI * BPG], I32, tag="nib")
            nc.vector.tensor_scalar(lo[:, :], qt[:, :], 15, None, op0=op.bitwise_and)
            nc.vector.tensor_scalar(
                hi[:, :], qt[:, :], 4, None, op0=op.logical_shift_right
            )

            wd = wdpool.tile([P, I], F16, tag="wd")
            wd4 = wd[:, :].rearrange("p (g b t) -> p g b t", g=GPI, b=BPG, t=2)
            a_b = at[:, :, None].broadcast_to([P, GPI, BPG])
            lo_r = lo[:, :].rearrange("p (g b) -> p g b", b=BPG)
            hi_r = hi[:, :].rearrange("p (g b) -> p g b", b=BPG)
            # W = (q - 7.5) * (2*norm/15)
            nc.vector.scalar_tensor_tensor(
                wd4[:, :, :, 0], lo_r, -7.5, a_b, op0=op.add, op1=op.mult
            )
            nc.vector.scalar_tensor_tensor(
                wd4[:, :, :, 1], hi_r, -7.5, a_b, op0=op.add, op1=op.mult
            )

            for k in range(KT):
                nc.sync.dma_start_transpose(
                    wT[:, k, ot * P : (ot + 1) * P], wd[:, k * P : (k + 1) * P]
                )

    # ---- phase 2: main matmul loop over token blocks ----------------------
    with (
        tc.tile_pool(name="xfpool", bufs=3) as xfpool,
        tc.tile_pool(name="xTpool", bufs=3) as xTpool,
        tc.tile_pool(name="opool", bufs=8) as opool,
        tc.tile_pool(name="pspool", bufs=8, space="PSUM") as pspool,
    ):
        for tb in range(TB):
            trow = slice(tb * P, (tb + 1) * P)
            # fp32 -> fp16 cast during the DMA (SWDGE)
            xf = xfpool.tile([P, I], F16, tag="xf")
            nc.gpsimd.dma_start(xf[:, :], x_ap[trow, :])
            xT = xTpool.tile([P, KT * P], F16, tag="xT")
            for k in range(KT):
                nc.sync.dma_start_transpose(
                    xT[:, k * P : (k + 1) * P], xf[:, k * P : (k + 1) * P]
                )

            ps = []
            for oc in range(NOC):
                pst = pspool.tile([P, OC], F32, tag="ps")
                ps.append(pst)
            for k in range(KT):
                lhs = xT[:, k * P : (k + 1) * P]
                for oc in range(NOC):
                    nc.tensor.matmul(
                        ps[oc][:, :],
                        lhs,
                        wT[:, k, oc * OC : (oc + 1) * OC],
                        start=(k == 0),
                        stop=(k == KT - 1),
                    )
            for oc in range(NOC):
                osb = opool.tile([P, OC], F32, tag="osb")
                nc.vector.scalar_tensor_tensor(
                    osb[:, :],
                    ps[oc][:, :],
                    0.0,
                    bias_rep[:, oc * OC : (oc + 1) * OC],
                    op0=op.add,
                    op1=op.add,
                )
                nc.sync.dma_start(y_ap[trow, oc * OC : (oc + 1) * OC], osb[:, :])

    singles.release()


def build_nc(T=T_FULL, I=IN_F, O=O_SH):
    nc = bacc.Bacc("TRN2", target_bir_lowering=False, debug=False)
    x = nc.dram_tensor("x", [T, I], F32, kind="ExternalInput")
    qw = nc.dram_tensor("qw", [O * I // GROUP, GROUP // 2], I32, kind="ExternalInput")
    wn = nc.dram_tensor("wn", [O * I // GROUP, 1], F32, kind="ExternalInput")
    b = nc.dram_tensor("bias", [O], F32, kind="ExternalInput")
    y = nc.dram_tensor("y", [T, O], F32, kind="ExternalOutput")
    with TileContext(nc) as tc:
        emit_linear4bit(tc, x.ap(), qw.ap(), wn.ap(), b.ap(), y.ap(), T, I, O)
    nc.compile()
    return nc


TRACE = False
LAST_RESULT = None


def kernel(input_tensor, quantized_weights, weight_normalization, bias):
    global LAST_RESULT
    from concourse.bass_utils import run_bass_kernel_spmd

    x = np.ascontiguousarray(
        np.asarray(input_tensor, dtype=np.float32).reshape(T_FULL, IN_F)
    )
    qw = np.asarray(quantized_weights, dtype=np.int32)
    wn = np.asarray(weight_normalization, dtype=np.float32)
    b = np.asarray(bias, dtype=np.float32)

    nc = build_nc()
    in_maps = []
    for c in range(N_CORES):
        in_maps.append(
            {
                "x": x,
                "qw": np.ascontiguousarray(qw[c * G_SH : (c + 1) * G_SH]),
                "wn": np.ascontiguousarray(wn[c * G_SH : (c + 1) * G_SH]),
                "bias": np.ascontiguousarray(b[c * O_SH : (c + 1) * O_SH]),
            }
        )
    res = run_bass_kernel_spmd(nc, in_maps, list(range(N_CORES)), trace=TRACE)
    LAST_RESULT = res
    y = np.concatenate([r["y"] for r in res.results], axis=1)
    return np.ascontiguousarray(y.reshape(B, S, OUT_F), dtype=np.float32)


# revision 11
# speedup vs baseline: 1.0700x; 1.0700x over previous
"""Trainium2 Bass kernel for a 4-bit-quantized linear layer.

Computes y = x @ W^T + bias where W [O, I] is dequantized on-chip from
packed int4 nibbles with a per-group (16 along I) abs-max scale:
    W[o, i] = (q/15) * 2*norm - norm = (2*norm/15) * (q - 7.5)

Distribution: tensor-parallel over output features. Each of the 8 cores
owns O/8 = 2048 output rows (its slice of quantized_weights /
weight_normalization / bias), the input is replicated, and the host
concatenates the per-core [T, 2048] outputs along the feature axis.

Per-core device program:
  1. Dequantize the weight shard to fp16 in [o, i] layout on DVE
     (bitwise nibble extraction + fused (q - 7.5) * a with a = 2*norm/15),
     then DMA-transpose to a resident [i, o] fp16 tensor in SBUF.
  2. Stream x in 128-token blocks: fp32->fp16 cast during the (SWDGE) DMA
     load, DMA-transpose to x^T k-tiles, and accumulate
     psum[t, o] += xT_k^T @ WT_k over the 32 k-tiles on the TensorEngine.
  3. Add bias (DVE, fused with the PSUM->SBUF copy) and DMA out.
"""

import numpy as np

import concourse.bass as bass
import concourse.mybir as mybir
from concourse import bacc
from concourse.tile import TileContext

# Full problem shapes (hardcoded; kernel.py must be self-contained).
B, S = 4, 2048
IN_F = 4096
OUT_F = 16384
GROUP = 16
N_CORES = 8
T_FULL = B * S                    # 8192 tokens
O_SH = OUT_F // N_CORES           # 2048 output features per core
G_SH = O_SH * IN_F // GROUP       # 524288 quant groups per core

F16 = mybir.dt.float16
F32 = mybir.dt.float32
I32 = mybir.dt.int32

P = 128  # partitions


def emit_linear4bit(tc, x_ap, qw_ap, wn_ap, bias_ap, y_ap, T, I, O):
    """Emit the per-core program into TileContext tc.

    x:  [T, I]  f32   (replicated input)
    qw: [O*I/16, 8] i32  (this core's group rows; each i32 holds one byte
                          = two nibbles)
    wn: [O*I/16, 1] f32  (per-group scale)
    bias: [O] f32
    y:  [T, O] f32
    """
    nc = tc.nc
    op = mybir.AluOpType

    KT = I // P                   # k tiles (contraction)
    OT = O // P                   # o tiles for dequant
    OC = min(512, O)              # psum chunk along o
    NOC = O // OC
    TB = T // P                   # token blocks
    GPI = I // GROUP              # groups per output row
    BPG = GROUP // 2              # bytes per group (8)

    # ---- resident tensors -------------------------------------------------
    # W^T, fp16, [i (partition within k-tile), k-tile, o] -> 2*KT*O bytes/part
    singles = tc.alloc_tile_pool(name="singles", bufs=1)
    wT = singles.tile([P, KT, O], F16, tag="wT", name="wT")
    bias_rep = singles.tile([P, O], F32, tag="bias_rep", name="bias_rep")

    # bias replicated across partitions via a broadcast-source DMA
    nc.sync.dma_start(bias_rep[:, :], bias_ap[None, :].broadcast_to([P, O]))

    # views of the weight inputs grouped by 128-row o tiles
    # qw rows: g = o * GPI + gi ; o = ot*128 + p
    qw_r = qw_ap.rearrange("(ot p g) b -> ot p (g b)", ot=OT, p=P, g=GPI)
    wn_r = wn_ap.rearrange("(ot p g) one -> ot p (g one)", ot=OT, p=P, g=GPI)

    # ---- phase 1: dequantize + transpose W --------------------------------
    with (
        tc.tile_pool(name="qpool", bufs=2) as qpool,
        tc.tile_pool(name="spool", bufs=2) as spool,
        tc.tile_pool(name="nibpool", bufs=2) as nibpool,
        tc.tile_pool(name="wdpool", bufs=2) as wdpool,
    ):
        for ot in range(OT):
            qt = qpool.tile([P, GPI * BPG], I32, tag="qt")
            nc.sync.dma_start(qt[:, :], qw_r[ot])
            wnt = spool.tile([P, GPI], F32, tag="wnt")
            nc.sync.dma_start(wnt[:, :], wn_r[ot])
            at = spool.tile([P, GPI], F32, tag="at")
            nc.vector.tensor_scalar_mul(at[:, :], wnt[:, :], 2.0 / 15.0)

            lo = nibpool.tile([P, GPI * BPG], I32, tag="nib")
            hi = nibpool.tile([P, GPI * BPG], I32, tag="nib")
            nc.vector.tensor_scalar(lo[:, :], qt[:, :], 15, None, op0=op.bitwise_and)
            nc.vector.tensor_scalar(
                hi[:, :], qt[:, :], 4, None, op0=op.logical_shift_right
            )

            wd = wdpool.tile([P, I], F16, tag="wd")
            wd4 = wd[:, :].rearrange("p (g b t) -> p g b t", g=GPI, b=BPG, t=2)
            a_b = at[:, :, None].broadcast_to([P, GPI, BPG])
            lo_r = lo[:, :].rearrange("p (g b) -> p g b", b=BPG)
            hi_r = hi[:, :].rearrange("p (g b) -> p g b", b=BPG)
            # W = (q - 7.5) * (2*norm/15)
            nc.vector.scalar_tensor_tensor(
                wd4[:, :, :, 0], lo_r, -7.5, a_b, op0=op.add, op1=op.mult
            )
            nc.vector.scalar_tensor_tensor(
                wd4[:, :, :, 1], hi_r, -7.5, a_b, op0=op.add, op1=op.mult
            )

            # one xbar-transpose for the whole o-tile: logical [I, 128] <-
            # [128, I]; out extra dim k folds into the partition dim
            nc.sync.dma_start_transpose(
                wT[:, :, ot * P : (ot + 1) * P], wd[:, :]
            )

    # ---- phase 2: main matmul loop over token blocks ----------------------
    with (
        tc.tile_pool(name="xfpool", bufs=3) as xfpool,
        tc.tile_pool(name="xTpool", bufs=3) as xTpool,
        tc.tile_pool(name="opool", bufs=8) as opool,
        tc.tile_pool(name="pspool", bufs=8, space="PSUM") as pspool,
    ):
        for tb in range(TB):
            trow = slice(tb * P, (tb + 1) * P)
            # fp32 -> fp16 cast during the DMA (SWDGE)
            xf = xfpool.tile([P, I], F16, tag="xf")
            nc.gpsimd.dma_start(xf[:, :], x_ap[trow, :])
            xT = xTpool.tile([P, KT, P], F16, tag="xT")
            nc.sync.dma_start_transpose(xT[:, :, :], xf[:, :])

            ps = []
            for oc in range(NOC):
                pst = pspool.tile([P, OC], F32, tag="ps")
                ps.append(pst)
            for k in range(KT):
                lhs = xT[:, k, :]
                for oc in range(NOC):
                    nc.tensor.matmul(
                        ps[oc][:, :],
                        lhs,
                        wT[:, k, oc * OC : (oc + 1) * OC],
                        start=(k == 0),
                        stop=(k == KT - 1),
                    )
            for oc in range(NOC):
                osb = opool.tile([P, OC], F32, tag="osb")
                nc.vector.scalar_tensor_tensor(
                    osb[:, :],
                    ps[oc][:, :],
                    0.0,
                    bias_rep[:, oc * OC : (oc + 1) * OC],
                    op0=op.add,
                    op1=op.add,
                )
                nc.sync.dma_start(y_ap[trow, oc * OC : (oc + 1) * OC], osb[:, :])

    singles.release()


def build_nc(T=T_FULL, I=IN_F, O=O_SH):
    nc = bacc.Bacc("TRN2", target_bir_lowering=False, debug=False)
    x = nc.dram_tensor("x", [T, I], F32, kind="ExternalInput")
    qw = nc.dram_tensor("qw", [O * I // GROUP, GROUP // 2], I32, kind="ExternalInput")
    wn = nc.dram_tensor("wn", [O * I // GROUP, 1], F32, kind="ExternalInput")
    b = nc.dram_tensor("bias", [O], F32, kind="ExternalInput")
    y = nc.dram_tensor("y", [T, O], F32, kind="ExternalOutput")
    with TileContext(nc) as tc:
        emit_linear4bit(tc, x.ap(), qw.ap(), wn.ap(), b.ap(), y.ap(), T, I, O)
    nc.compile()
    return nc


TRACE = False
LAST_RESULT = None


def kernel(input_tensor, quantized_weights, weight_normalization, bias):
    global LAST_RESULT
    from concourse.bass_utils import run_bass_kernel_spmd

    x = np.ascontiguousarray(
        np.asarray(input_tensor, dtype=np.float32).reshape(T_FULL, IN_F)
    )
    qw = np.asarray(quantized_weights, dtype=np.int32)
    wn = np.asarray(weight_normalization, dtype=np.float32)
    b = np.asarray(bias, dtype=np.float32)

    nc = build_nc()
    in_maps = []
    for c in range(N_CORES):
        in_maps.append(
            {
                "x": x,
                "qw": np.ascontiguousarray(qw[c * G_SH : (c + 1) * G_SH]),
                "wn": np.ascontiguousarray(wn[c * G_SH : (c + 1) * G_SH]),
                "bias": np.ascontiguousarray(b[c * O_SH : (c + 1) * O_SH]),
            }
        )
    res = run_bass_kernel_spmd(nc, in_maps, list(range(N_CORES)), trace=TRACE)
    LAST_RESULT = res
    y = np.concatenate([r["y"] for r in res.results], axis=1)
    return np.ascontiguousarray(y.reshape(B, S, OUT_F), dtype=np.float32)


# revision 12
# speedup vs baseline: 40.9319x; 38.2533x over previous
"""Trainium2 Bass kernel for a 4-bit-quantized linear layer.

Computes y = x @ W^T + bias where W [O, I] is dequantized on-chip from
packed int4 nibbles with a per-group (16 along I) abs-max scale:
    W[o, i] = (q/15) * 2*norm - norm = (2*norm/15) * (q - 7.5)

Distribution: tensor-parallel over output features. Each of the 8 cores
owns O/8 = 2048 output rows (its slice of quantized_weights /
weight_normalization / bias), the input is replicated, and the host
concatenates the per-core [T, 2048] outputs along the feature axis.

Per-core device program:
  1. Dequantize the weight shard to fp16 in [o, i] layout on DVE
     (bitwise nibble extraction + fused (q - 7.5) * a with a = 2*norm/15),
     then one xbar DMA-transpose per 128-row o-tile into a resident
     [i, o] fp16 tensor in SBUF (3D out AP folds the k-tile dim into the
     partition dim, so the per-op HWDGE fixed cost is paid 16x, not 512x).
  2. Stream x in 128-token blocks: fp32->fp16 cast during the (SWDGE) DMA
     load, one xbar DMA-transpose to all 32 x^T k-tiles, and accumulate
     psum[t, o] += xT_k^T @ WT_k over the 32 k-tiles on the TensorEngine
     (x^T k-tile stationary, 4 psum banks of 512 output features each).
  3. Add bias (DVE scalar_tensor_tensor fused with the PSUM->SBUF copy)
     and DMA out.
"""

import numpy as np

import concourse.bass as bass
import concourse.mybir as mybir
from concourse import bacc
from concourse.tile import TileContext

# Full problem shapes (hardcoded; kernel.py must be self-contained).
B, S = 4, 2048
IN_F = 4096
OUT_F = 16384
GROUP = 16
N_CORES = 8
T_FULL = B * S                    # 8192 tokens
O_SH = OUT_F // N_CORES           # 2048 output features per core
G_SH = O_SH * IN_F // GROUP       # 524288 quant groups per core

F16 = mybir.dt.float16
F32 = mybir.dt.float32
I32 = mybir.dt.int32

P = 128  # partitions


def emit_linear4bit(tc, x_ap, qw_ap, wn_ap, bias_ap, y_ap, T, I, O):
    """Emit the per-core program into TileContext tc.

    x:  [T, I]  f32   (replicated input)
    qw: [O*I/16, 8] i32  (this core's group rows; each i32 holds one byte
                          = two nibbles)
    wn: [O*I/16, 1] f32  (per-group scale)
    bias: [O] f32
    y:  [T, O] f32
    """
    nc = tc.nc
    op = mybir.AluOpType

    KT = I // P                   # k tiles (contraction)
    OT = O // P                   # o tiles for dequant
    OC = min(512, O)              # psum chunk along o
    NOC = O // OC
    TB = T // P                   # token blocks
    GPI = I // GROUP              # groups per output row
    BPG = GROUP // 2              # bytes per group (8)

    # ---- resident tensors -------------------------------------------------
    # W^T, fp16, [i (partition within k-tile), k-tile, o] -> 2*KT*O bytes/part
    singles = tc.alloc_tile_pool(name="singles", bufs=1)
    wT = singles.tile([P, KT, O], F16, tag="wT", name="wT")
    bias_rep = singles.tile([P, O], F32, tag="bias_rep", name="bias_rep")

    # bias replicated across partitions via a broadcast-source DMA
    nc.sync.dma_start(bias_rep[:, :], bias_ap[None, :].broadcast_to([P, O]))

    # views of the weight inputs grouped by 128-row o tiles
    # qw rows: g = o * GPI + gi ; o = ot*128 + p
    qw_r = qw_ap.rearrange("(ot p g) b -> ot p (g b)", ot=OT, p=P, g=GPI)
    wn_r = wn_ap.rearrange("(ot p g) one -> ot p (g one)", ot=OT, p=P, g=GPI)

    # ---- phase 1: dequantize + transpose W --------------------------------
    with (
        tc.tile_pool(name="qpool", bufs=2) as qpool,
        tc.tile_pool(name="spool", bufs=2) as spool,
        tc.tile_pool(name="nibpool", bufs=2) as nibpool,
        tc.tile_pool(name="wdpool", bufs=2) as wdpool,
    ):
        for ot in range(OT):
            qt = qpool.tile([P, GPI * BPG], I32, tag="qt")
            nc.sync.dma_start(qt[:, :], qw_r[ot])
            wnt = spool.tile([P, GPI], F32, tag="wnt")
            nc.sync.dma_start(wnt[:, :], wn_r[ot])
            at = spool.tile([P, GPI], F32, tag="at")
            nc.vector.tensor_scalar_mul(at[:, :], wnt[:, :], 2.0 / 15.0)

            lo = nibpool.tile([P, GPI * BPG], I32, tag="nib")
            hi = nibpool.tile([P, GPI * BPG], I32, tag="nib")
            nc.vector.tensor_scalar(lo[:, :], qt[:, :], 15, None, op0=op.bitwise_and)
            nc.vector.tensor_scalar(
                hi[:, :], qt[:, :], 4, None, op0=op.logical_shift_right
            )

            wd = wdpool.tile([P, I], F16, tag="wd")
            wd4 = wd[:, :].rearrange("p (g b t) -> p g b t", g=GPI, b=BPG, t=2)
            a_b = at[:, :, None].broadcast_to([P, GPI, BPG])
            lo_r = lo[:, :].rearrange("p (g b) -> p g b", b=BPG)
            hi_r = hi[:, :].rearrange("p (g b) -> p g b", b=BPG)
            # W = (q - 7.5) * (2*norm/15)
            nc.vector.scalar_tensor_tensor(
                wd4[:, :, :, 0], lo_r, -7.5, a_b, op0=op.add, op1=op.mult
            )
            nc.vector.scalar_tensor_tensor(
                wd4[:, :, :, 1], hi_r, -7.5, a_b, op0=op.add, op1=op.mult
            )

            # one xbar-transpose for the whole o-tile: logical [I, 128] <-
            # [128, I]; out extra dim k folds into the partition dim
            nc.sync.dma_start_transpose(
                wT[:, :, ot * P : (ot + 1) * P], wd[:, :]
            )

    # ---- phase 2: main matmul loop over token blocks ----------------------
    with (
        tc.tile_pool(name="xfpool", bufs=3) as xfpool,
        tc.tile_pool(name="xTpool", bufs=3) as xTpool,
        tc.tile_pool(name="opool", bufs=8) as opool,
        tc.tile_pool(name="pspool", bufs=8, space="PSUM") as pspool,
    ):
        for tb in range(TB):
            trow = slice(tb * P, (tb + 1) * P)
            # fp32 -> fp16 cast during the DMA (SWDGE)
            xf = xfpool.tile([P, I], F16, tag="xf")
            nc.gpsimd.dma_start(xf[:, :], x_ap[trow, :])
            xT = xTpool.tile([P, KT, P], F16, tag="xT")
            nc.sync.dma_start_transpose(xT[:, :, :], xf[:, :])

            ps = []
            for oc in range(NOC):
                pst = pspool.tile([P, OC], F32, tag="ps")
                ps.append(pst)
            for k in range(KT):
                lhs = xT[:, k, :]
                for oc in range(NOC):
                    nc.tensor.matmul(
                        ps[oc][:, :],
                        lhs,
                        wT[:, k, oc * OC : (oc + 1) * OC],
                        start=(k == 0),
                        stop=(k == KT - 1),
                    )
            for oc in range(NOC):
                osb = opool.tile([P, OC], F32, tag="osb")
                nc.vector.scalar_tensor_tensor(
                    osb[:, :],
                    ps[oc][:, :],
                    0.0,
                    bias_rep[:, oc * OC : (oc + 1) * OC],
                    op0=op.add,
                    op1=op.add,
                )
                nc.sync.dma_start(y_ap[trow, oc * OC : (oc + 1) * OC], osb[:, :])

    singles.release()


def build_nc(T=T_FULL, I=IN_F, O=O_SH):
    nc = bacc.Bacc("TRN2", target_bir_lowering=False, debug=False)
    x = nc.dram_tensor("x", [T, I], F32, kind="ExternalInput")
    qw = nc.dram_tensor("qw", [O * I // GROUP, GROUP // 2], I32, kind="ExternalInput")
    wn = nc.dram_tensor("wn", [O * I // GROUP, 1], F32, kind="ExternalInput")
    b = nc.dram_tensor("bias", [O], F32, kind="ExternalInput")
    y = nc.dram_tensor("y", [T, O], F32, kind="ExternalOutput")
    with TileContext(nc) as tc:
        emit_linear4bit(tc, x.ap(), qw.ap(), wn.ap(), b.ap(), y.ap(), T, I, O)
    nc.compile()
    return nc


TRACE = False
LAST_RESULT = None


def kernel(input_tensor, quantized_weights, weight_normalization, bias):
    global LAST_RESULT
    from concourse.bass_utils import run_bass_kernel_spmd

    x = np.ascontiguousarray(
        np.asarray(input_tensor, dtype=np.float32).reshape(T_FULL, IN_F)
    )
    qw = np.asarray(quantized_weights, dtype=np.int32)
    wn = np.asarray(weight_normalization, dtype=np.float32)
    b = np.asarray(bias, dtype=np.float32)

    nc = build_nc()
    in_maps = []
    for c in range(N_CORES):
        in_maps.append(
            {
                "x": x,
                "qw": np.ascontiguousarray(qw[c * G_SH : (c + 1) * G_SH]),
                "wn": np.ascontiguousarray(wn[c * G_SH : (c + 1) * G_SH]),
                "bias": np.ascontiguousarray(b[c * O_SH : (c + 1) * O_SH]),
            }
        )
    res = run_bass_kernel_spmd(nc, in_maps, list(range(N_CORES)), trace=TRACE)
    LAST_RESULT = res
    y = np.concatenate([r["y"] for r in res.results], axis=1)
    return np.ascontiguousarray(y.reshape(B, S, OUT_F), dtype=np.float32)
